# revision 26
# baseline (speedup 1.0000x reference)
"""Trainium2 Bass kernel for a 2-layer GAT encoder (nn_Encoder_63273458205283).

v3: partition-aligned edge layout + chunked dma_gather.  See memory note
trn2-gat-gather-limits for the HW constraints this encodes.
Precision: tables bf16 (gather bandwidth), but the layer-1 ELU output and the
whole W2 chain (transpose, matmul, correction add) run in f32 so table2 rows
are rounded only once.
"""

import os
from dataclasses import dataclass, field

import numpy as np

N = 50000
E = 800000
IN = 128
H = 2
C1 = 128
C2 = 64
NEG_SLOPE = 0.2
NCORES = 8
R1 = 384          # layer1 table row elements (768B bf16)
R2 = 256          # layer2 table row elements (512B bf16)
AS1, AD1 = 256, 258
AS2, AD2 = 128, 130
KC = 8            # dma_gather chunk size in tiles (1024 idx ring limit)
EBIAS_PAD = -10000.0


@dataclass
class Cfg:
    n_cores: int = NCORES
    n_nodes: int = N
    in_dim: int = IN
    c1: int = C1
    c2: int = C2
    wpc: int = 49
    u0: int = 7
    dt_bf16: bool = True
    J: list = field(default_factory=list)
    K: list = field(default_factory=list)

    @property
    def spc(self):
        return self.wpc * 128

    @property
    def n_slots(self):
        return self.n_cores * self.spc

    @property
    def half_slots(self):
        return self.n_cores // 2 * self.spc      # 25088


def prep(cfg: Cfg, x, edge_index, W1, att_src1, att_dst1, b1, W2, att_src2,
         att_dst2, b2):
    import ml_dtypes
    np_dt = ml_dtypes.bfloat16 if cfg.dt_bf16 else np.float32

    nn = cfg.n_nodes
    src = np.asarray(edge_index[0], dtype=np.int64)
    dst = np.asarray(edge_index[1], dtype=np.int64)
    loop = np.arange(nn, dtype=np.int64)
    src = np.concatenate([src, loop])
    dst = np.concatenate([dst, loop])

    # ---- round 1: region split (A -> cores 0-3, B -> cores 4-7)
    deg = np.bincount(dst, minlength=nn)
    order = np.argsort(-deg, kind="stable")
    inA = np.zeros(nn, dtype=bool)
    inA[order[0::2]] = True
    src_inA = inA[src]
    lo = np.bincount(dst[src_inA], minlength=nn)
    hi = deg - lo

    # ---- round 2: within each half, sort by (lo, hi) desc, pack windows
    half = cfg.half_slots
    win_per_half = half // 128                   # 196
    rank_count = cfg.wpc
    pi = np.empty(nn, dtype=np.int64)
    win_nodes = {}
    for hf, mask in ((0, inA), (1, ~inA)):
        nodes = np.where(mask)[0]
        key = np.lexsort((-hi[nodes], -lo[nodes]))
        nodes = nodes[key]
        for w in range(win_per_half):
            mem = nodes[w * 128:(w + 1) * 128]
            core = hf * 4 + w % 4
            rank = w // 4
            base = core * cfg.spc + rank * 128
            pi[mem] = base + np.arange(len(mem))
            win_nodes[(core, rank)] = (mem, base)

    Jr = np.zeros(rank_count, dtype=np.int64)
    Hr = np.zeros(rank_count, dtype=np.int64)
    for (core, rank), (mem, base) in win_nodes.items():
        if len(mem):
            Jr[rank] = max(Jr[rank], lo[mem].max())
        Hr[rank] = max(Hr[rank], hi[mem].max() if len(mem) else 0, 1)
    Jr = np.maximum(Jr, 1)
    cfg.J = [int(j) for j in Jr]
    cfg.K = [int(j + h) for j, h in zip(Jr, Hr)]

    # ---- edge slot arrays
    esrc_slot = pi[src]
    edst_slot = pi[dst]
    e_order = np.argsort(edst_slot, kind="stable")
    esrc_slot = esrc_slot[e_order]
    edst_slot = edst_slot[e_order]
    elo = esrc_slot < half
    sub = np.lexsort((~elo, edst_slot))
    esrc_slot = esrc_slot[sub]
    elo = elo[sub]
    edst_slot = edst_slot[sub]
    starts = np.searchsorted(edst_slot, np.arange(cfg.n_slots))
    lo_counts = np.bincount(edst_slot[elo], minlength=cfg.n_slots)
    within = np.arange(len(esrc_slot)) - starts[edst_slot]
    slot_rank = (edst_slot % cfg.spc) // 128
    tJ = Jr[slot_rank]
    t_idx = np.where(within < lo_counts[edst_slot], within,
                     tJ + (within - lo_counts[edst_slot]))

    Ksum = int(sum(cfg.K))
    idx_cols = sum(((j + KC - 1) // KC * KC + (k - j + KC - 1) // KC * KC)
                   for j, k in zip(cfg.J, cfg.K)) * 8
    in_maps_idx = np.zeros((cfg.n_cores, 128, idx_cols), dtype=np.int16)
    in_maps_eb = np.full((cfg.n_cores, 128, 2 * Ksum), EBIAS_PAD,
                         dtype=np.float32)

    srcflat = np.full((cfg.n_cores, rank_count, 128, int(max(cfg.K))), -1,
                      dtype=np.int64)
    core_of_slot = edst_slot // cfg.spc
    p_of_slot = edst_slot % 128
    srcflat[core_of_slot, slot_rank, p_of_slot, t_idx] = esrc_slot
    filled = np.zeros(cfg.n_slots, dtype=bool)
    filled[pi] = True
    ghost = np.where(~filled)[0]
    g_core = ghost // cfg.spc
    g_rank = (ghost % cfg.spc) // 128
    g_p = ghost % 128
    g_t = np.where(ghost < half, 0, Jr[g_rank])
    srcflat[g_core, g_rank, g_p, g_t] = ghost

    def wrap16(idx1d):
        a = idx1d.reshape(-1, 16).T.astype(np.int16)
        return np.tile(a, (8, 1))

    eb_off = 0
    col_off = 0
    rank_idx_off = []
    rank_eb_off = []
    for i in range(rank_count):
        J, K = cfg.J[i], cfg.K[i]
        rank_idx_off.append(col_off)
        rank_eb_off.append(eb_off)
        for c in range(cfg.n_cores):
            sf = srcflat[c, i]
            real = sf[:, :K] >= 0
            eb = np.where(real, 0.0, EBIAS_PAD).astype(np.float32)
            in_maps_eb[c, :, eb_off:eb_off + K] = eb
            in_maps_eb[c, :, Ksum + eb_off:Ksum + eb_off + K] = eb
            co = col_off
            for part, t0, t1 in (("lo", 0, J), ("hi", J, K)):
                nt = t1 - t0
                for c0 in range(0, nt, KC):
                    cc = min(KC, nt - c0)
                    tt = sf[:, t0 + c0:t0 + c0 + cc]
                    iv = np.where(tt >= 0,
                                  tt - (half if part == "hi" else 0), 0)
                    flat = np.ascontiguousarray(iv.T).reshape(-1)
                    w = wrap16(flat)
                    in_maps_idx[c, :, co:co + cc * 8] = w
                    co += cc * 8
        eb_off += K
        col_off = co
    cfg.rank_idx_off = rank_idx_off
    cfg.rank_eb_off = rank_eb_off
    cfg.ksum = Ksum
    cfg.idx_cols = idx_cols

    # ---- features / weights
    x = np.asarray(x, dtype=np.float32)
    x_perm = np.zeros((cfg.n_slots, cfg.in_dim), dtype=np.float32)
    x_perm[pi] = x[:nn]
    xT = np.ascontiguousarray(x_perm.T).astype(np_dt)

    W1 = np.asarray(W1, np.float32)
    W2 = np.asarray(W2, np.float32)
    a_s1 = np.asarray(att_src1, np.float32)
    a_d1 = np.asarray(att_dst1, np.float32)
    a_s2 = np.asarray(att_src2, np.float32)
    a_d2 = np.asarray(att_dst2, np.float32)
    b1 = np.asarray(b1, np.float32)
    b2 = np.asarray(b2, np.float32)

    W1h = W1.reshape(cfg.in_dim, H, cfg.c1)
    wext1 = np.zeros((cfg.in_dim, R1), dtype=np.float32)
    wext1[:, 0:cfg.c1] = W1h[:, 0]
    wext1[:, cfg.c1:2 * cfg.c1] = W1h[:, 1]
    wext1[:, AS1:AS1 + 2] = np.einsum("khc,hc->kh", W1h, a_s1)
    wext1[:, AD1:AD1 + 2] = np.einsum("khc,hc->kh", W1h, a_d1)
    wext1 = wext1.astype(np_dt)

    W2h = W2.reshape(2 * cfg.c1, H, cfg.c2)
    w2full = np.zeros((2 * cfg.c1, R2), dtype=np.float32)
    w2full[:, 0:cfg.c2] = W2h[:, 0]
    w2full[:, cfg.c2:2 * cfg.c2] = W2h[:, 1]
    w2full[:, AS2:AS2 + 2] = np.einsum("khc,hc->kh", W2h, a_s2)
    w2full[:, AD2:AD2 + 2] = np.einsum("khc,hc->kh", W2h, a_d2)
    # W2 chain runs in f32 on-device
    w2ext = np.ascontiguousarray(w2full.reshape(2, cfg.c1, R2)).astype(
        np.float32)

    corr1 = np.tile(((b1 - 1.0) @ w2full)[None, :], (128, 1)).astype(
        np.float32)
    corr2 = np.tile((b2 - 1.0)[None, :], (128, 1)).astype(np.float32)
    ident = np.eye(128, dtype=np.float32)                    # f32 transpose
    identkh = np.zeros((128, KC, 2, 128), dtype=np.float32)
    identkh[np.arange(128), :, :, np.arange(128)] = 1.0
    identkh = identkh.astype(np_dt)

    in_maps = []
    for c in range(cfg.n_cores):
        in_maps.append({
            "xT": np.ascontiguousarray(xT[:, c * cfg.spc:(c + 1) * cfg.spc]),
            "wext1": wext1,
            "w2ext": w2ext,
            "corr1": corr1,
            "corr2": corr2,
            "ident": ident,
            "identkh": identkh,
            "idx": in_maps_idx[c],
            "ebias": in_maps_eb[c].astype(np_dt),
        })
    return in_maps, pi


def build(cfg: Cfg):
    import concourse.bass as bass
    import concourse.bacc as bacc
    import concourse.mybir as mybir
    import concourse.tile as tile
    from concourse import library_config
    from concourse.bass import ds

    f32 = mybir.dt.float32
    DT = mybir.dt.bfloat16 if cfg.dt_bf16 else mybir.dt.float32
    i16 = mybir.dt.int16
    Alu = mybir.AluOpType
    Act = mybir.ActivationFunctionType
    ET = mybir.EngineType

    n_slots, spc, half = cfg.n_slots, cfg.spc, cfg.half_slots
    c1, c2 = cfg.c1, cfg.c2
    Ksum, idx_cols = cfg.ksum, cfg.idx_cols

    nc = bacc.Bacc(num_devices=cfg.n_cores, num_swdge_queues=4)

    xT_d = nc.dram_tensor("xT", [cfg.in_dim, spc], DT, kind="ExternalInput")
    wext1_d = nc.dram_tensor("wext1", [cfg.in_dim, R1], DT,
                             kind="ExternalInput")
    w2ext_d = nc.dram_tensor("w2ext", [2, c1, R2], f32, kind="ExternalInput")
    corr1_d = nc.dram_tensor("corr1", [128, R2], f32, kind="ExternalInput")
    corr2_d = nc.dram_tensor("corr2", [128, 2 * c2], f32,
                             kind="ExternalInput")
    ident_d = nc.dram_tensor("ident", [128, 128], f32, kind="ExternalInput")
    identkh_d = nc.dram_tensor("identkh", [128, KC, 2, 128], DT,
                               kind="ExternalInput")
    idx_d = nc.dram_tensor("idx", [128, idx_cols], i16, kind="ExternalInput")
    ebias_d = nc.dram_tensor("ebias", [128, 2 * Ksum], DT,
                             kind="ExternalInput")
    out2_d = nc.dram_tensor("out2", [spc, 2 * c2], f32, kind="ExternalOutput")

    t1shard = nc.dram_tensor("t1shard", [spc, R1], DT, kind="Internal")
    table1 = nc.dram_tensor("table1", [n_slots, R1], DT, kind="Internal")
    h2shard = nc.dram_tensor("h2shard", [spc, R2], DT, kind="Internal")
    h2table = nc.dram_tensor("h2table", [n_slots, R2], DT, kind="Internal")

    hint = (ET.DVE, ET.PE, ET.Activation)

    with tile.TileContext(nc) as tc:
        with (
            tc.tile_pool(name="const", bufs=1) as cpool,
            tc.tile_pool(name="work", bufs=3) as wpool,
            tc.tile_pool(name="small", bufs=3) as spool,
            tc.tile_pool(name="psum", bufs=2, space="PSUM") as ppool,
        ):
            nc.gpsimd.load_library(library_config.mlp)
            wext1_sb = cpool.tile([cfg.in_dim, R1], DT, tag="wext1")
            nc.sync.dma_start(wext1_sb[:], wext1_d[:, :])
            w2ext_sb = cpool.tile([c1, 2, R2], f32, tag="w2ext")
            nc.sync.dma_start(
                w2ext_sb[:], w2ext_d[:, :, :].rearrange("b p c -> p b c"))
            corr1_sb = cpool.tile([128, R2], f32, tag="corr1")
            nc.sync.dma_start(corr1_sb[:], corr1_d[:, :])
            corr2_sb = cpool.tile([128, 2 * c2], f32, tag="corr2")
            nc.sync.dma_start(corr2_sb[:], corr2_d[:, :])
            ident_sb = cpool.tile([128, 128], f32, tag="ident")
            nc.sync.dma_start(ident_sb[:], ident_d[:, :])
            identkh_sb = cpool.tile([128, KC, 2, 128], DT, tag="identkh")
            nc.sync.dma_start(identkh_sb[:], identkh_d[:, :, :, :])

            # ---- phase 0
            if "0" not in os.environ.get("GAT_SKIP", ""):
                u0 = cfg.u0
                with tc.For_i(0, spc, u0 * 128, hint_engines=hint) as i0:
                    xsl = wpool.tile([cfg.in_dim, u0 * 128], DT, tag="xsl")
                    nc.sync.dma_start(xsl[:], xT_d[:, ds(i0, u0 * 128)])
                    rsl = wpool.tile([128, u0, R1], DT, tag="rsl")
                    for t in range(u0):
                        ps0 = ppool.tile([128, R1], f32, tag="ps0", bufs=2)
                        nc.tensor.matmul(ps0[:],
                                         lhsT=xsl[:, t * 128:(t + 1) * 128],
                                         rhs=wext1_sb[:], start=True,
                                         stop=True)
                        nc.vector.tensor_copy(rsl[:, t, :], ps0[:])
                    nc.sync.dma_start(
                        t1shard[ds(i0, u0 * 128), :].rearrange(
                            "(u p) c -> p u c", p=128), rsl[:])
                nc.gpsimd.collective_compute(
                    kind="AllGather", op=mybir.AluOpType.bypass,
                    replica_groups=[list(range(cfg.n_cores))],
                    ins=[t1shard[:, :]], outs=[table1[0:n_slots, :]])

            # ---- edge phase
            def edge_phase(table, shard, R, C, as_off, ad_off, finish):
                tabA = table[0:half, :]
                tabB = table[half:n_slots, :]
                gq = [0]
                for i in range(cfg.wpc):
                    J, K = cfg.J[i], cfg.K[i]
                    nch = (J + KC - 1) // KC + (K - J + KC - 1) // KC
                    io = cfg.rank_idx_off[i]
                    eo = cfg.rank_eb_off[i]
                    adwin = spool.tile([128, 2], DT, tag="adwin")
                    nc.sync.dma_start(
                        adwin[:],
                        shard[i * 128:(i + 1) * 128, ad_off:ad_off + 2])
                    ebs = wpool.tile([128, 2, K], DT, tag="ebs")
                    nc.sync.dma_start(
                        ebs[:, 0, :], ebias_d[:, eo:eo + K])
                    nc.sync.dma_start(
                        ebs[:, 1, :], ebias_d[:, Ksum + eo:Ksum + eo + K])
                    den_all = spool.tile([128, 2, nch], f32, tag="den")
                    acc0 = ppool.tile([128, C], f32, tag="acc0", bufs=2)
                    acc1 = ppool.tile([128, C], f32, tag="acc1", bufs=2)
                    acc = (acc0, acc1)
                    chunks = ([("A", t0, min(KC, J - t0))
                               for t0 in range(0, J, KC)] +
                              [("B", t0, min(KC, K - t0))
                               for t0 in range(J, K, KC)])
                    idxr = wpool.tile([128, K * 8], i16, tag="idxs",
                                      bufs=3)
                    nc.sync.dma_start(idxr[:], idx_d[:, io:io + K * 8])
                    col = 0
                    ci = 0
                    for (ab, t0, cc) in chunks:
                        idxs = idxr[:, col:col + cc * 8]
                        col += cc * 8
                        gath = wpool.tile([128, cc, R], DT, tag="gath",
                                          bufs=7)
                        nc.gpsimd.dma_gather(
                            gath[:], tabA if ab == "A" else tabB, idxs,
                            cc * 128, cc * 128, R,
                            queue_num=gq[0] % 4)
                        gq[0] += 1
                        e1 = spool.tile([128, 2, cc], f32, tag="e1")
                        nc.vector.tensor_tensor(
                            out=e1[:],
                            in0=gath[:, :, as_off:as_off + 2].rearrange(
                                "p k h -> p h k"),
                            in1=adwin[:, :].to_broadcast([128, 2, cc]),
                            op=Alu.add)
                        e2 = spool.tile([128, 2, cc], f32, tag="e2")
                        nc.vector.tensor_tensor(
                            out=e2[:], in0=e1[:],
                            in1=ebs[:, :, t0:t0 + cc], op=Alu.add)
                        lr = spool.tile([128, 2, cc], f32, tag="lr")
                        nc.vector.scalar_tensor_tensor(
                            out=lr[:], in0=e2[:], scalar=NEG_SLOPE,
                            in1=e2[:], op0=Alu.mult, op1=Alu.max)
                        ex = spool.tile([128, 2, cc], DT, tag="ex")
                        for h in range(2):
                            nc.scalar.activation(
                                out=ex[:, h, :], in_=lr[:, h, :],
                                func=Act.Exp,
                                accum_out=den_all[:, h, ci:ci + 1])
                        diag = wpool.tile([128, cc, 2, 128], DT, tag="diag",
                                          bufs=4)
                        nc.vector.tensor_tensor(
                            out=diag[:], in0=identkh_sb[:, 0:cc, :, :],
                            in1=ex[:, :, :].rearrange(
                                "p h k -> p k h").to_broadcast(
                                    [128, cc, 2, 128]),
                            op=Alu.mult)
                        for k in range(cc):
                            for h in range(2):
                                nc.tensor.matmul(
                                    acc[h][:],
                                    lhsT=diag[:, k, h, :],
                                    rhs=gath[:, k, h * C:(h + 1) * C],
                                    start=(ci == 0 and k == 0),
                                    stop=(ci == nch - 1 and k == cc - 1))
                        ci += 1
                    den = spool.tile([128, 2], f32, tag="dent")
                    nc.vector.tensor_reduce(
                        out=den[:], in_=den_all[:], axis=mybir.AxisListType.X,
                        op=Alu.add)
                    recip = spool.tile([128, 2], f32, tag="recip")
                    nc.vector.reciprocal(recip[:], den[:])
                    ob = spool.tile([128, 2, C], f32, tag="ob")
                    for h in range(2):
                        nc.scalar.activation(
                            out=ob[:, h, :], in_=acc[h][:],
                            func=Act.Copy, scale=recip[:, h:h + 1])
                    tmin = spool.tile([128, 2 * C], f32, tag="tmin")
                    nc.vector.tensor_scalar(
                        out=tmin[:], in0=ob[:, :, :], scalar1=0.0,
                        scalar2=None, op0=Alu.min)
                    em = spool.tile([128, 2 * C], f32, tag="em")
                    nc.scalar.activation(out=em[:], in_=tmin[:], func=Act.Exp)
                    ee = spool.tile([128, 2 * C], f32, tag="ee")
                    nc.vector.scalar_tensor_tensor(
                        out=ee[:], in0=ob[:, :, :], scalar=0.0,
                        in1=em[:], op0=Alu.max, op1=Alu.add)
                    finish.emit(i, ee)

            class Fin1:
                def emit(self, i, ee):
                    h1T = []
                    for b in range(2):
                        pst = ppool.tile([128, 128], f32, tag="pst", bufs=1)
                        nc.tensor.transpose(
                            pst[:], ee[:, b * 128:(b + 1) * 128], ident_sb[:])
                        ht = wpool.tile([128, 128], f32, tag=f"h1T{b}")
                        nc.scalar.activation(out=ht[:], in_=pst[:],
                                             func=Act.Copy)
                        h1T.append(ht)
                    h2p = ppool.tile([128, R2], f32, tag="h2p", bufs=1)
                    nc.tensor.matmul(h2p[:], lhsT=h1T[0][:],
                                     rhs=w2ext_sb[:, 0, :], start=True,
                                     stop=False)
                    nc.tensor.matmul(h2p[:], lhsT=h1T[1][:],
                                     rhs=w2ext_sb[:, 1, :], start=False,
                                     stop=True)
                    osl = wpool.tile([128, R2], DT, tag="osl1")
                    nc.vector.tensor_tensor(out=osl[:], in0=h2p[:],
                                            in1=corr1_sb[:], op=Alu.add)
                    nc.sync.dma_start(h2shard[i * 128:(i + 1) * 128, :],
                                      osl[:])

            class Fin2:
                def emit(self, i, ee):
                    osl = wpool.tile([128, 2 * c2], f32, tag="osl2")
                    nc.vector.tensor_tensor(out=osl[:], in0=ee[:],
                                            in1=corr2_sb[:], op=Alu.add)
                    nc.sync.dma_start(out2_d[i * 128:(i + 1) * 128, :],
                                      osl[:])

            if "1" not in os.environ.get("GAT_SKIP", ""):
                edge_phase(table1, t1shard, R1, c1, AS1, AD1, Fin1())
            if "c" not in os.environ.get("GAT_SKIP", ""):
                nc.gpsimd.collective_compute(
                    kind="AllGather", op=mybir.AluOpType.bypass,
                    replica_groups=[list(range(cfg.n_cores))],
                    ins=[h2shard[:, :]], outs=[h2table[0:n_slots, :]])
            if "2" not in os.environ.get("GAT_SKIP", ""):
                edge_phase(h2table, h2shard, R2, c2, AS2, AD2, Fin2())

    nc.finalize()
    return nc


_CACHE: dict = {}


def kernel(x, edge_index, W1, att_src1, att_dst1, b1, W2, att_src2, att_dst2,
           b2):
    from concourse.bass_utils import run_bass_kernel_spmd

    cfg = Cfg(dt_bf16=bool(int(os.environ.get("GAT_BF16", "1"))))
    in_maps, pi = prep(cfg, x, edge_index, W1, att_src1, att_dst1, b1, W2,
                       att_src2, att_dst2, b2)
    key = (tuple(cfg.J), tuple(cfg.K), cfg.dt_bf16)
    if key not in _CACHE:
        _CACHE[key] = build(cfg)
    nc = _CACHE[key]
    res = run_bass_kernel_spmd(nc, in_maps, core_ids=list(range(cfg.n_cores)))
    out = np.concatenate([res.results[c]["out2"] for c in range(cfg.n_cores)],
                         axis=0)
    h2 = np.ascontiguousarray(out[pi[:cfg.n_nodes]], dtype=np.float32)
    encoded = np.asarray(x, dtype=np.float32)
    return (h2, encoded)


# revision 27
# speedup vs baseline: 1.0221x; 1.0221x over previous
"""Trainium2 Bass kernel for a 2-layer GAT encoder (nn_Encoder_63273458205283).

v3: partition-aligned edge layout + chunked dma_gather.  See memory note
trn2-gat-gather-limits for the HW constraints this encodes.
Precision: tables bf16 (gather bandwidth), but the layer-1 ELU output and the
whole W2 chain (transpose, matmul, correction add) run in f32 so table2 rows
are rounded only once.
"""

import os
from dataclasses import dataclass, field

import numpy as np

N = 50000
E = 800000
IN = 128
H = 2
C1 = 128
C2 = 64
NEG_SLOPE = 0.2
NCORES = 8
R1 = 384          # layer1 table row elements (768B bf16)
R2 = 256          # layer2 table row elements (512B bf16)
AS1, AD1 = 256, 258
AS2, AD2 = 128, 130
KC = 8            # dma_gather chunk size in tiles (1024 idx ring limit)
EBIAS_PAD = -10000.0


@dataclass
class Cfg:
    n_cores: int = NCORES
    n_nodes: int = N
    in_dim: int = IN
    c1: int = C1
    c2: int = C2
    wpc: int = 49
    u0: int = 7
    dt_bf16: bool = True
    J: list = field(default_factory=list)
    K: list = field(default_factory=list)

    @property
    def spc(self):
        return self.wpc * 128

    @property
    def n_slots(self):
        return self.n_cores * self.spc

    @property
    def half_slots(self):
        return self.n_cores // 2 * self.spc      # 25088


def prep(cfg: Cfg, x, edge_index, W1, att_src1, att_dst1, b1, W2, att_src2,
         att_dst2, b2):
    import ml_dtypes
    np_dt = ml_dtypes.bfloat16 if cfg.dt_bf16 else np.float32

    nn = cfg.n_nodes
    src = np.asarray(edge_index[0], dtype=np.int64)
    dst = np.asarray(edge_index[1], dtype=np.int64)
    loop = np.arange(nn, dtype=np.int64)
    src = np.concatenate([src, loop])
    dst = np.concatenate([dst, loop])

    # ---- round 1: region split (A -> cores 0-3, B -> cores 4-7)
    deg = np.bincount(dst, minlength=nn)
    order = np.argsort(-deg, kind="stable")
    inA = np.zeros(nn, dtype=bool)
    inA[order[0::2]] = True
    src_inA = inA[src]
    lo = np.bincount(dst[src_inA], minlength=nn)
    hi = deg - lo

    # ---- round 2: within each half, sort by (lo, hi) desc, pack windows
    half = cfg.half_slots
    win_per_half = half // 128                   # 196
    rank_count = cfg.wpc
    pi = np.empty(nn, dtype=np.int64)
    win_nodes = {}
    for hf, mask in ((0, inA), (1, ~inA)):
        nodes = np.where(mask)[0]
        key = np.lexsort((-hi[nodes], -lo[nodes]))
        nodes = nodes[key]
        for w in range(win_per_half):
            mem = nodes[w * 128:(w + 1) * 128]
            core = hf * 4 + w % 4
            rank = w // 4
            base = core * cfg.spc + rank * 128
            pi[mem] = base + np.arange(len(mem))
            win_nodes[(core, rank)] = (mem, base)

    Jr = np.zeros(rank_count, dtype=np.int64)
    Hr = np.zeros(rank_count, dtype=np.int64)
    for (core, rank), (mem, base) in win_nodes.items():
        if len(mem):
            Jr[rank] = max(Jr[rank], lo[mem].max())
        Hr[rank] = max(Hr[rank], hi[mem].max() if len(mem) else 0, 1)
    Jr = np.maximum(Jr, 1)
    cfg.J = [int(j) for j in Jr]
    cfg.K = [int(j + h) for j, h in zip(Jr, Hr)]

    # ---- edge slot arrays
    esrc_slot = pi[src]
    edst_slot = pi[dst]
    e_order = np.argsort(edst_slot, kind="stable")
    esrc_slot = esrc_slot[e_order]
    edst_slot = edst_slot[e_order]
    elo = esrc_slot < half
    sub = np.lexsort((~elo, edst_slot))
    esrc_slot = esrc_slot[sub]
    elo = elo[sub]
    edst_slot = edst_slot[sub]
    starts = np.searchsorted(edst_slot, np.arange(cfg.n_slots))
    lo_counts = np.bincount(edst_slot[elo], minlength=cfg.n_slots)
    within = np.arange(len(esrc_slot)) - starts[edst_slot]
    slot_rank = (edst_slot % cfg.spc) // 128
    tJ = Jr[slot_rank]
    t_idx = np.where(within < lo_counts[edst_slot], within,
                     tJ + (within - lo_counts[edst_slot]))

    Ksum = int(sum(cfg.K))
    idx_cols = sum(((j + KC - 1) // KC * KC + (k - j + KC - 1) // KC * KC)
                   for j, k in zip(cfg.J, cfg.K)) * 8
    in_maps_idx = np.zeros((cfg.n_cores, 128, idx_cols), dtype=np.int16)
    in_maps_eb = np.full((cfg.n_cores, 128, 2 * Ksum), EBIAS_PAD,
                         dtype=np.float32)

    srcflat = np.full((cfg.n_cores, rank_count, 128, int(max(cfg.K))), -1,
                      dtype=np.int64)
    core_of_slot = edst_slot // cfg.spc
    p_of_slot = edst_slot % 128
    srcflat[core_of_slot, slot_rank, p_of_slot, t_idx] = esrc_slot
    filled = np.zeros(cfg.n_slots, dtype=bool)
    filled[pi] = True
    ghost = np.where(~filled)[0]
    g_core = ghost // cfg.spc
    g_rank = (ghost % cfg.spc) // 128
    g_p = ghost % 128
    g_t = np.where(ghost < half, 0, Jr[g_rank])
    srcflat[g_core, g_rank, g_p, g_t] = ghost

    def wrap16(idx1d):
        a = idx1d.reshape(-1, 16).T.astype(np.int16)
        return np.tile(a, (8, 1))

    eb_off = 0
    col_off = 0
    rank_idx_off = []
    rank_eb_off = []
    for i in range(rank_count):
        J, K = cfg.J[i], cfg.K[i]
        rank_idx_off.append(col_off)
        rank_eb_off.append(eb_off)
        for c in range(cfg.n_cores):
            sf = srcflat[c, i]
            real = sf[:, :K] >= 0
            eb = np.where(real, 0.0, EBIAS_PAD).astype(np.float32)
            in_maps_eb[c, :, eb_off:eb_off + K] = eb
            in_maps_eb[c, :, Ksum + eb_off:Ksum + eb_off + K] = eb
            co = col_off
            for part, t0, t1 in (("lo", 0, J), ("hi", J, K)):
                nt = t1 - t0
                for c0 in range(0, nt, KC):
                    cc = min(KC, nt - c0)
                    tt = sf[:, t0 + c0:t0 + c0 + cc]
                    iv = np.where(tt >= 0,
                                  tt - (half if part == "hi" else 0), 0)
                    flat = np.ascontiguousarray(iv.T).reshape(-1)
                    w = wrap16(flat)
                    in_maps_idx[c, :, co:co + cc * 8] = w
                    co += cc * 8
        eb_off += K
        col_off = co
    cfg.rank_idx_off = rank_idx_off
    cfg.rank_eb_off = rank_eb_off
    cfg.ksum = Ksum
    cfg.idx_cols = idx_cols

    # ---- features / weights
    x = np.asarray(x, dtype=np.float32)
    x_perm = np.zeros((cfg.n_slots, cfg.in_dim), dtype=np.float32)
    x_perm[pi] = x[:nn]
    xT = np.ascontiguousarray(x_perm.T).astype(np_dt)

    W1 = np.asarray(W1, np.float32)
    W2 = np.asarray(W2, np.float32)
    a_s1 = np.asarray(att_src1, np.float32)
    a_d1 = np.asarray(att_dst1, np.float32)
    a_s2 = np.asarray(att_src2, np.float32)
    a_d2 = np.asarray(att_dst2, np.float32)
    b1 = np.asarray(b1, np.float32)
    b2 = np.asarray(b2, np.float32)

    W1h = W1.reshape(cfg.in_dim, H, cfg.c1)
    wext1 = np.zeros((cfg.in_dim, R1), dtype=np.float32)
    wext1[:, 0:cfg.c1] = W1h[:, 0]
    wext1[:, cfg.c1:2 * cfg.c1] = W1h[:, 1]
    wext1[:, AS1:AS1 + 2] = np.einsum("khc,hc->kh", W1h, a_s1)
    wext1[:, AD1:AD1 + 2] = np.einsum("khc,hc->kh", W1h, a_d1)
    wext1 = wext1.astype(np_dt)

    W2h = W2.reshape(2 * cfg.c1, H, cfg.c2)
    w2full = np.zeros((2 * cfg.c1, R2), dtype=np.float32)
    w2full[:, 0:cfg.c2] = W2h[:, 0]
    w2full[:, cfg.c2:2 * cfg.c2] = W2h[:, 1]
    w2full[:, AS2:AS2 + 2] = np.einsum("khc,hc->kh", W2h, a_s2)
    w2full[:, AD2:AD2 + 2] = np.einsum("khc,hc->kh", W2h, a_d2)
    # W2 chain runs in f32 on-device
    w2ext = np.ascontiguousarray(w2full.reshape(2, cfg.c1, R2)).astype(
        np.float32)

    corr1 = np.tile(((b1 - 1.0) @ w2full)[None, :], (128, 1)).astype(
        np.float32)
    corr2 = np.tile((b2 - 1.0)[None, :], (128, 1)).astype(np.float32)
    ident = np.eye(128, dtype=np.float32)                    # f32 transpose
    identkh = np.zeros((128, KC, 2, 128), dtype=np.float32)
    identkh[np.arange(128), :, :, np.arange(128)] = 1.0
    identkh = identkh.astype(np_dt)

    in_maps = []
    for c in range(cfg.n_cores):
        in_maps.append({
            "xT": np.ascontiguousarray(xT[:, c * cfg.spc:(c + 1) * cfg.spc]),
            "wext1": wext1,
            "w2ext": w2ext,
            "corr1": corr1,
            "corr2": corr2,
            "ident": ident,
            "identkh": identkh,
            "idx": in_maps_idx[c],
            "ebias": in_maps_eb[c].astype(np_dt),
        })
    return in_maps, pi


def build(cfg: Cfg):
    import concourse.bass as bass
    import concourse.bacc as bacc
    import concourse.mybir as mybir
    import concourse.tile as tile
    from concourse import library_config
    from concourse.bass import ds

    f32 = mybir.dt.float32
    DT = mybir.dt.bfloat16 if cfg.dt_bf16 else mybir.dt.float32
    i16 = mybir.dt.int16
    Alu = mybir.AluOpType
    Act = mybir.ActivationFunctionType
    ET = mybir.EngineType

    n_slots, spc, half = cfg.n_slots, cfg.spc, cfg.half_slots
    c1, c2 = cfg.c1, cfg.c2
    Ksum, idx_cols = cfg.ksum, cfg.idx_cols

    nc = bacc.Bacc(num_devices=cfg.n_cores, num_swdge_queues=4)

    xT_d = nc.dram_tensor("xT", [cfg.in_dim, spc], DT, kind="ExternalInput")
    wext1_d = nc.dram_tensor("wext1", [cfg.in_dim, R1], DT,
                             kind="ExternalInput")
    w2ext_d = nc.dram_tensor("w2ext", [2, c1, R2], f32, kind="ExternalInput")
    corr1_d = nc.dram_tensor("corr1", [128, R2], f32, kind="ExternalInput")
    corr2_d = nc.dram_tensor("corr2", [128, 2 * c2], f32,
                             kind="ExternalInput")
    ident_d = nc.dram_tensor("ident", [128, 128], f32, kind="ExternalInput")
    identkh_d = nc.dram_tensor("identkh", [128, KC, 2, 128], DT,
                               kind="ExternalInput")
    idx_d = nc.dram_tensor("idx", [128, idx_cols], i16, kind="ExternalInput")
    ebias_d = nc.dram_tensor("ebias", [128, 2 * Ksum], DT,
                             kind="ExternalInput")
    out2_d = nc.dram_tensor("out2", [spc, 2 * c2], f32, kind="ExternalOutput")

    t1shard = nc.dram_tensor("t1shard", [spc, R1], DT, kind="Internal")
    table1 = nc.dram_tensor("table1", [n_slots, R1], DT, kind="Internal")
    h2shard = nc.dram_tensor("h2shard", [spc, R2], DT, kind="Internal")
    h2table = nc.dram_tensor("h2table", [n_slots, R2], DT, kind="Internal")

    hint = (ET.DVE, ET.PE, ET.Activation)

    with tile.TileContext(nc) as tc:
        with (
            tc.tile_pool(name="const", bufs=1) as cpool,
            tc.tile_pool(name="work", bufs=3) as wpool,
            tc.tile_pool(name="small", bufs=3) as spool,
            tc.tile_pool(name="psum", bufs=2, space="PSUM") as ppool,
        ):
            nc.gpsimd.load_library(library_config.mlp)
            wext1_sb = cpool.tile([cfg.in_dim, R1], DT, tag="wext1")
            nc.sync.dma_start(wext1_sb[:], wext1_d[:, :])
            w2ext_sb = cpool.tile([c1, 2, R2], f32, tag="w2ext")
            nc.sync.dma_start(
                w2ext_sb[:], w2ext_d[:, :, :].rearrange("b p c -> p b c"))
            corr1_sb = cpool.tile([128, R2], f32, tag="corr1")
            nc.sync.dma_start(corr1_sb[:], corr1_d[:, :])
            corr2_sb = cpool.tile([128, 2 * c2], f32, tag="corr2")
            nc.sync.dma_start(corr2_sb[:], corr2_d[:, :])
            ident_sb = cpool.tile([128, 128], f32, tag="ident")
            nc.sync.dma_start(ident_sb[:], ident_d[:, :])
            identkh_sb = cpool.tile([128, KC, 2, 128], DT, tag="identkh")
            nc.sync.dma_start(identkh_sb[:], identkh_d[:, :, :, :])

            # ---- phase 0
            if "0" not in os.environ.get("GAT_SKIP", ""):
                u0 = cfg.u0
                with tc.For_i(0, spc, u0 * 128, hint_engines=hint) as i0:
                    xsl = wpool.tile([cfg.in_dim, u0 * 128], DT, tag="xsl")
                    nc.sync.dma_start(xsl[:], xT_d[:, ds(i0, u0 * 128)])
                    rsl = wpool.tile([128, u0, R1], DT, tag="rsl")
                    for t in range(u0):
                        ps0 = ppool.tile([128, R1], f32, tag="ps0", bufs=2)
                        nc.tensor.matmul(ps0[:],
                                         lhsT=xsl[:, t * 128:(t + 1) * 128],
                                         rhs=wext1_sb[:], start=True,
                                         stop=True)
                        nc.vector.tensor_copy(rsl[:, t, :], ps0[:])
                    nc.sync.dma_start(
                        t1shard[ds(i0, u0 * 128), :].rearrange(
                            "(u p) c -> p u c", p=128), rsl[:])
                nc.gpsimd.collective_compute(
                    kind="AllGather", op=mybir.AluOpType.bypass,
                    replica_groups=[list(range(cfg.n_cores))],
                    ins=[t1shard[:, :]], outs=[table1[0:n_slots, :]])

            # ---- edge phase
            def edge_phase(table, shard, R, C, as_off, ad_off, finish):
                tabA = table[0:half, :]
                tabB = table[half:n_slots, :]
                for i in range(cfg.wpc):
                    J, K = cfg.J[i], cfg.K[i]
                    nch = (J + KC - 1) // KC + (K - J + KC - 1) // KC
                    io = cfg.rank_idx_off[i]
                    eo = cfg.rank_eb_off[i]
                    adwin = spool.tile([128, 2], DT, tag="adwin")
                    nc.sync.dma_start(
                        adwin[:],
                        shard[i * 128:(i + 1) * 128, ad_off:ad_off + 2])
                    ebs = wpool.tile([128, 2, K], DT, tag="ebs")
                    nc.sync.dma_start(
                        ebs[:, 0, :], ebias_d[:, eo:eo + K])
                    nc.sync.dma_start(
                        ebs[:, 1, :], ebias_d[:, Ksum + eo:Ksum + eo + K])
                    den_all = spool.tile([128, 2, nch], f32, tag="den")
                    acc0 = ppool.tile([128, C], f32, tag="acc0", bufs=2)
                    acc1 = ppool.tile([128, C], f32, tag="acc1", bufs=2)
                    acc = (acc0, acc1)
                    chunks = ([("A", t0, min(KC, J - t0))
                               for t0 in range(0, J, KC)] +
                              [("B", t0, min(KC, K - t0))
                               for t0 in range(J, K, KC)])
                    col = io
                    ci = 0
                    for (ab, t0, cc) in chunks:
                        idxs = wpool.tile([128, cc * 8], i16, tag="idxs",
                                          bufs=7)
                        nc.sync.dma_start(idxs[:],
                                          idx_d[:, col:col + cc * 8])
                        col += cc * 8
                        gath = wpool.tile([128, cc, R], DT, tag="gath",
                                          bufs=7)
                        nc.gpsimd.dma_gather(
                            gath[:], tabA if ab == "A" else tabB, idxs[:],
                            cc * 128, cc * 128, R,
                            queue_num=ci % 4)
                        e1 = spool.tile([128, 2, cc], f32, tag="e1")
                        nc.vector.tensor_tensor(
                            out=e1[:],
                            in0=gath[:, :, as_off:as_off + 2].rearrange(
                                "p k h -> p h k"),
                            in1=adwin[:, :].to_broadcast([128, 2, cc]),
                            op=Alu.add)
                        e2 = spool.tile([128, 2, cc], f32, tag="e2")
                        nc.vector.tensor_tensor(
                            out=e2[:], in0=e1[:],
                            in1=ebs[:, :, t0:t0 + cc], op=Alu.add)
                        lr = spool.tile([128, 2, cc], f32, tag="lr")
                        nc.vector.scalar_tensor_tensor(
                            out=lr[:], in0=e2[:], scalar=NEG_SLOPE,
                            in1=e2[:], op0=Alu.mult, op1=Alu.max)
                        ex = spool.tile([128, 2, cc], DT, tag="ex")
                        for h in range(2):
                            nc.scalar.activation(
                                out=ex[:, h, :], in_=lr[:, h, :],
                                func=Act.Exp,
                                accum_out=den_all[:, h, ci:ci + 1])
                        diag = wpool.tile([128, cc, 2, 128], DT, tag="diag",
                                          bufs=4)
                        nc.vector.tensor_tensor(
                            out=diag[:], in0=identkh_sb[:, 0:cc, :, :],
                            in1=ex[:, :, :].rearrange(
                                "p h k -> p k h").to_broadcast(
                                    [128, cc, 2, 128]),
                            op=Alu.mult)
                        for k in range(cc):
                            for h in range(2):
                                nc.tensor.matmul(
                                    acc[h][:],
                                    lhsT=diag[:, k, h, :],
                                    rhs=gath[:, k, h * C:(h + 1) * C],
                                    start=(ci == 0 and k == 0),
                                    stop=(ci == nch - 1 and k == cc - 1))
                        ci += 1
                    den = spool.tile([128, 2], f32, tag="dent")
                    nc.vector.tensor_reduce(
                        out=den[:], in_=den_all[:], axis=mybir.AxisListType.X,
                        op=Alu.add)
                    recip = spool.tile([128, 2], f32, tag="recip")
                    nc.vector.reciprocal(recip[:], den[:])
                    ob = spool.tile([128, 2, C], f32, tag="ob")
                    for h in range(2):
                        nc.scalar.activation(
                            out=ob[:, h, :], in_=acc[h][:],
                            func=Act.Copy, scale=recip[:, h:h + 1])
                    tmin = spool.tile([128, 2 * C], f32, tag="tmin")
                    nc.vector.tensor_scalar(
                        out=tmin[:], in0=ob[:, :, :], scalar1=0.0,
                        scalar2=None, op0=Alu.min)
                    em = spool.tile([128, 2 * C], f32, tag="em")
                    nc.scalar.activation(out=em[:], in_=tmin[:], func=Act.Exp)
                    ee = spool.tile([128, 2 * C], f32, tag="ee")
                    nc.vector.scalar_tensor_tensor(
                        out=ee[:], in0=ob[:, :, :], scalar=0.0,
                        in1=em[:], op0=Alu.max, op1=Alu.add)
                    finish.emit(i, ee)

            class Fin1:
                def emit(self, i, ee):
                    h1T = []
                    for b in range(2):
                        pst = ppool.tile([128, 128], f32, tag="pst", bufs=1)
                        nc.tensor.transpose(
                            pst[:], ee[:, b * 128:(b + 1) * 128], ident_sb[:])
                        ht = wpool.tile([128, 128], f32, tag=f"h1T{b}")
                        nc.scalar.activation(out=ht[:], in_=pst[:],
                                             func=Act.Copy)
                        h1T.append(ht)
                    h2p = ppool.tile([128, R2], f32, tag="h2p", bufs=1)
                    nc.tensor.matmul(h2p[:], lhsT=h1T[0][:],
                                     rhs=w2ext_sb[:, 0, :], start=True,
                                     stop=False)
                    nc.tensor.matmul(h2p[:], lhsT=h1T[1][:],
                                     rhs=w2ext_sb[:, 1, :], start=False,
                                     stop=True)
                    osl = wpool.tile([128, R2], DT, tag="osl1")
                    nc.vector.tensor_tensor(out=osl[:], in0=h2p[:],
                                            in1=corr1_sb[:], op=Alu.add)
                    nc.sync.dma_start(h2shard[i * 128:(i + 1) * 128, :],
                                      osl[:])

            class Fin2:
                def emit(self, i, ee):
                    osl = wpool.tile([128, 2 * c2], f32, tag="osl2")
                    nc.vector.tensor_tensor(out=osl[:], in0=ee[:],
                                            in1=corr2_sb[:], op=Alu.add)
                    nc.sync.dma_start(out2_d[i * 128:(i + 1) * 128, :],
                                      osl[:])

            if "1" not in os.environ.get("GAT_SKIP", ""):
                edge_phase(table1, t1shard, R1, c1, AS1, AD1, Fin1())
            if "c" not in os.environ.get("GAT_SKIP", ""):
                nc.gpsimd.collective_compute(
                    kind="AllGather", op=mybir.AluOpType.bypass,
                    replica_groups=[list(range(cfg.n_cores))],
                    ins=[h2shard[:, :]], outs=[h2table[0:n_slots, :]])
            if "2" not in os.environ.get("GAT_SKIP", ""):
                edge_phase(h2table, h2shard, R2, c2, AS2, AD2, Fin2())

    nc.finalize()
    return nc


_CACHE: dict = {}


def kernel(x, edge_index, W1, att_src1, att_dst1, b1, W2, att_src2, att_dst2,
           b2):
    from concourse.bass_utils import run_bass_kernel_spmd

    cfg = Cfg(dt_bf16=bool(int(os.environ.get("GAT_BF16", "1"))))
    in_maps, pi = prep(cfg, x, edge_index, W1, att_src1, att_dst1, b1, W2,
                       att_src2, att_dst2, b2)
    key = (tuple(cfg.J), tuple(cfg.K), cfg.dt_bf16)
    if key not in _CACHE:
        _CACHE[key] = build(cfg)
    nc = _CACHE[key]
    res = run_bass_kernel_spmd(nc, in_maps, core_ids=list(range(cfg.n_cores)))
    out = np.concatenate([res.results[c]["out2"] for c in range(cfg.n_cores)],
                         axis=0)
    h2 = np.ascontiguousarray(out[pi[:cfg.n_nodes]], dtype=np.float32)
    encoded = np.asarray(x, dtype=np.float32)
    return (h2, encoded)
